# revision 1
# baseline (speedup 1.0000x reference)
"""Trainium2 Bass kernel for nn_MeshDownConv (2-layer SplineConv GNN).

Zero-communication sharding: 8 cores = 4 meshes x 2 halves. Each core
redundantly computes layer 1 for the FULL mesh (both halves, 392 windows)
so no cross-core collective or synchronization is needed anywhere; layer 2
computes only the core's own half (first 196 windows). All per-core data
uses an "own-half-first" row ordering so one static SPMD program serves
all cores: node n's table row on core h is (sigma(n) - h*NHALF) mod NN.

Per window the device pipeline is the incidence-matmul design: dma_gather
of source rows (256B granule), DVE builds u[e,(k,c)] = basis_k[e]*xj[e,c]
and the one-hot incidence inc[e,n] = (dstrel[e]==n), PE contracts the
edge dim into PSUM (z[n,(k,c)] += inc^T @ u), then the node side
multiplies by Wflat, adds root term + bias, and applies relu. Layer-1
rows are written straight into the local full table (tfull) that layer 2
gathers from.

Gather indices are streamed from DRAM in supergroups (not SBUF-resident),
and each group's gather is split into two 2304-index pieces so SWDGE
descriptor generation overlaps SDMA drain in the ring.
"""
import sys

sys.path.insert(0, "/opt/trn_rl_repo")

import numpy as np

import concourse.bass as bass
import concourse.mybir as mybir
from concourse import bacc, tile, bass_utils

F32 = mybir.dt.float32
I16 = mybir.dt.int16
I32 = mybir.dt.int32


class CFG:
    C = 32            # in channels
    O = 32            # out channels
    KK = 9            # spline kernels

    @property
    def RL(self):
        # table row length: 256B granule for dma_gather
        return 64 if self.DT == F32 else 128
    NW = 196          # windows per half
    NCHA = 9          # chunks (of 128 edges) per window per pass
    GW = 4            # windows per gather group
    SG = 8            # groups per idx-streaming supergroup
    NSPLIT = 2        # gather pieces per (group, pass)
    N = 50000         # real nodes per mesh
    E = 800000        # edges per mesh
    B = 4             # meshes
    NCORES = 8
    DT = mybir.dt.float16   # edge-side dtype (table/xj/u/inc/basis)
    UPOOL = 0               # u-build chunks offloaded to gpsimd (of NCHA)
    INCPOOL = False         # build incidence on gpsimd (else DVE)

    @property
    def NCH(self):
        return 2 * self.NCHA

    @property
    def NHALF(self):
        return self.NW * 128

    @property
    def NN(self):
        return 2 * self.NHALF

    @property
    def NWT(self):
        # total windows per core (layer 1 covers the full mesh)
        return 2 * self.NW

    @property
    def NCHT(self):
        return self.NWT * self.NCH

    @property
    def NSLOTT(self):
        # gather slots per core per pass (layer 1, full mesh)
        return self.NWT * self.NCHA * 128

    @property
    def NI16(self):
        return self.NSLOTT // 16


def _np_dt(dt):
    return {F32: np.float32, mybir.dt.float16: np.float16}[dt]


# ----------------------------------------------------------------- host prep

def _quad_basis_np(t):
    return np.stack([0.5 * (1.0 - t) ** 2, -t * t + t + 0.5, 0.5 * t * t],
                    axis=-1)


def _balance_nodes(deg, nbins, cap_nodes=128):
    """Greedy: assign nodes (desc by degree) to the lightest non-full bin.
    Returns sigma: old -> new id (bin*128 + slot)."""
    import heapq
    n = deg.shape[0]
    order = np.argsort(-deg, kind="stable")
    heap = [(0, b) for b in range(nbins)]
    heapq.heapify(heap)
    counts = np.zeros(nbins, np.int64)
    sums = np.zeros(nbins, np.int64)
    sigma = np.empty(n, np.int64)
    for old in order:
        while True:
            s, b = heapq.heappop(heap)
            if counts[b] < cap_nodes:
                break
        sigma[old] = b * 128 + counts[b]
        counts[b] += 1
        sums[b] += deg[old]
        if counts[b] < cap_nodes:
            heapq.heappush(heap, (sums[b], b))
    return sigma, sums


def _host_prep_mesh(cfg, x, edge, pseudo):
    """Per-mesh host preprocessing.

    Returns (tab0 [NN, RL] DT in global sigma order, cores, sigma) where
    cores[h] = (IDXA [128, NI16] i16, IDXB, ED [128, NCHT, 10] DT) in the
    own-half-first row ordering of core h.
    """
    npdt = _np_dt(cfg.DT)
    src, dst = edge[0].astype(np.int64), edge[1].astype(np.int64)
    E = src.shape[0]

    B0 = _quad_basis_np(pseudo[:, 0].astype(np.float32))
    B1 = _quad_basis_np(pseudo[:, 1].astype(np.float32))
    basis = (B1[:, :, None] * B0[:, None, :]).reshape(E, cfg.KK)

    deg = np.bincount(dst, minlength=cfg.N)
    sigma, sums = _balance_nodes(deg, 2 * cfg.NW)

    gsrc = sigma[src]
    gdst = sigma[dst]

    tab0 = np.zeros((cfg.NN, cfg.RL), npdt)
    tab0[sigma[np.arange(cfg.N)], :cfg.C] = x.astype(npdt)

    capa = cfg.NCHA * 128
    cores = []
    for h in range(2):
        srow = (gsrc - h * cfg.NHALF) % cfg.NN
        drow = (gdst - h * cfg.NHALF) % cfg.NN
        win = drow // 128
        p = (srow >= cfg.NHALF).astype(np.int64)
        key = win * 2 + p
        order = np.argsort(key, kind="stable")
        starts = np.searchsorted(key[order], np.arange(2 * cfg.NWT + 1))
        IDX = np.zeros((2, 16, cfg.NI16), np.int16)
        ED = np.zeros((128, cfg.NCHT, 1 + cfg.KK), np.float32)
        for w in range(cfg.NWT):
            for pp in range(2):
                lo, hi = starts[2 * w + pp], starts[2 * w + pp + 1]
                eids = order[lo:hi]
                cnt = hi - lo
                assert cnt <= capa, (cnt, capa, w, pp)
                slot = np.arange(cnt)
                ch = (pp * cfg.NCHA + w * cfg.NCH) + slot // 128
                lane = slot % 128
                ED[lane, ch, 0] = (drow[eids] - w * 128).astype(np.float32)
                ED[lane, ch, 1:] = basis[eids]
                gslot = (w * cfg.NCHA + slot // 128) * 128 + lane
                sidx = srow[eids] - pp * cfg.NHALF
                IDX[pp, gslot % 16, gslot // 16] = sidx.astype(np.int16)
        cores.append((np.tile(IDX[0], (8, 1)), np.tile(IDX[1], (8, 1)),
                      ED.astype(npdt)))
    return tab0, cores, sigma


# ------------------------------------------------------------- bass program

def _ap(t, offset, pattern):
    base = t if isinstance(t, bass.AP) else t[:]
    return bass.AP(base.tensor, base.offset + offset, pattern)


def _sap(t, offset, freedims, npart=None):
    """Slice of an SBUF/PSUM tile in the flat AP convention."""
    base = t if isinstance(t, bass.AP) else t[:]
    p = base.ap[0]
    part = [p[0], p[1] if npart is None else npart]
    return bass.AP(base.tensor, base.offset + offset, [part] + list(freedims))


def build_program(cfg, ncores, nogather=False, nocompute=False,
                  upool=None, repeat=1):
    nc = bacc.Bacc("TRN2", target_bir_lowering=False, debug=False,
                   num_devices=ncores)
    DT = cfg.DT
    C, O, KK, RL = cfg.C, cfg.O, cfg.KK, cfg.RL
    NCH, NCHA, GW, NW = cfg.NCH, cfg.NCHA, cfg.GW, cfg.NW
    NG = NW // GW           # groups per half
    UD = KK * C             # 288
    upool_eff = cfg.UPOOL if upool is None else upool

    tab0 = nc.dram_tensor("tab0", [cfg.NN, RL], DT, kind="ExternalInput")
    idxa = nc.dram_tensor("idxa", [128, cfg.NI16], I16, kind="ExternalInput")
    idxb = nc.dram_tensor("idxb", [128, cfg.NI16], I16, kind="ExternalInput")
    edd = nc.dram_tensor("edd", [128, cfg.NCHT, 1 + KK], DT,
                         kind="ExternalInput")
    wf1 = nc.dram_tensor("wf1", [96, 96], F32, kind="ExternalInput")
    wf2 = nc.dram_tensor("wf2", [96, 96], F32, kind="ExternalInput")
    rt1 = nc.dram_tensor("rt1", [C, O], DT, kind="ExternalInput")
    rt2 = nc.dram_tensor("rt2", [C, O], DT, kind="ExternalInput")
    bb1 = nc.dram_tensor("bb1", [O, 1], F32, kind="ExternalInput")
    bb2 = nc.dram_tensor("bb2", [O, 1], F32, kind="ExternalInput")
    outt = nc.dram_tensor("out", [cfg.NHALF, C], F32, kind="ExternalOutput")

    with tile.TileContext(nc, num_cores=ncores) as tc:
        with tc.tile_pool(name="const", bufs=1) as cpool, \
             tc.tile_pool(name="dram", bufs=1, space="DRAM") as dpool, \
             tc.tile_pool(name="work", bufs=2) as wpool, \
             tc.tile_pool(name="psum", bufs=1, space="PSUM") as ppool:

            from concourse import library_config
            nc.gpsimd.load_library(library_config.mlp)

            # ---- constants
            iotai = cpool.tile([128, 128], I32, name="iotai")
            nc.gpsimd.iota(iotai[:], pattern=[[1, 128]], base=0,
                           channel_multiplier=0)
            iotaf = cpool.tile([128, 128], DT, name="iotaf")
            nc.vector.tensor_copy(iotaf[:], iotai[:])
            idximp = cpool.tile([128, 128], I32, name="idximp")
            nc.gpsimd.iota(idximp[:], pattern=[[1, 128]], base=0,
                           channel_multiplier=-1)
            identf = cpool.tile([128, 128], F32, name="identf")
            nc.vector.tensor_scalar(out=identf[:], in0=idximp[:],
                                    scalar1=0, scalar2=None,
                                    op0=mybir.AluOpType.is_equal)
            identd = identf
            if DT != F32:
                identd = cpool.tile([128, 128], DT, name="identd")
                nc.vector.tensor_copy(identd[:], identf[:])

            wfs = [cpool.tile([96, 96], F32, name=f"wfs{i}") for i in range(2)]
            rts = [cpool.tile([C, O], DT, name=f"rts{i}") for i in range(2)]
            bbs = [cpool.tile([O, 1], F32, name=f"bbs{i}") for i in range(2)]
            for i, (wsrc, rsrc, bsrc) in enumerate(
                    [(wf1, rt1, bb1), (wf2, rt2, bb2)]):
                nc.sync.dma_start(wfs[i][:], wsrc[:])
                nc.sync.dma_start(rts[i][:], rsrc[:])
                nc.sync.dma_start(bbs[i][:], bsrc[:])

            # ---- internal dram: local full table written by layer 1
            tfull = dpool.tile([cfg.NN, RL], DT, name="tfull")

            xjfix = None
            if nogather:
                xjfix = [cpool.tile([128, GW * NCHA * RL], DT,
                                    name=f"xjfix{p}") for p in range(2)]
                for p in range(2):
                    nc.vector.memset(xjfix[p][:], 0)

            nsg = GW * NCHA * 128          # gather idxs per (group, pass)
            npiece = nsg // cfg.NSPLIT
            nchp = GW * NCHA // cfg.NSPLIT  # chunks per gather piece

            def layer(tabfull, xown, wfsb, rtsb, bbsb, rows_out, last, ng):
                for g0 in range(0, ng, cfg.SG):
                    gcnt = min(cfg.SG, ng - g0)
                    # stream this supergroup's gather indices (both passes)
                    sidxt = [None, None]
                    for p, srct in ((0, idxa), (1, idxb)):
                        st = wpool.tile([128, cfg.SG * nsg // 16], I16,
                                        name=f"sidx{p}", bufs=2)
                        nc.sync.dma_start(
                            _sap(st, 0, [[1, gcnt * nsg // 16]]),
                            _ap(srct.ap(), g0 * nsg // 16,
                                [[cfg.NI16, 128], [1, gcnt * nsg // 16]]))
                        sidxt[p] = st
                    for gl in range(gcnt):
                        g = g0 + gl
                        xjt = [None, None]
                        for p in range(2):
                            if nogather:
                                xjt[p] = xjfix[p]
                                continue
                            xj = wpool.tile([128, GW * NCHA * RL], DT,
                                            name=f"xj{p}", bufs=2)
                            for s in range(cfg.NSPLIT):
                                # custom-DMA SBUF APs use the flat
                                # convention: partition step = row length
                                nc.gpsimd.dma_gather(
                                    out_ap=_sap(xj, s * nchp * RL,
                                                [[RL, nchp], [1, RL]]),
                                    in_ap=_ap(tabfull,
                                              p * cfg.NHALF * RL,
                                              [[RL, cfg.NHALF], [1, RL]]),
                                    idxs_ap=_sap(
                                        sidxt[p],
                                        (gl * nsg + s * npiece) // 16,
                                        [[1, npiece // 16]]),
                                    num_idxs=npiece,
                                    num_idxs_reg=npiece,
                                    elem_size=RL,
                                    single_packet=False,
                                )
                            xjt[p] = xj
                        edt = wpool.tile([128, GW * NCH * (1 + KK)], DT,
                                         name="edt", bufs=2)
                        nc.sync.dma_start(
                            edt[:],
                            _ap(edd.ap(), g * GW * NCH * (1 + KK),
                                [[cfg.NCHT * (1 + KK), 128],
                                 [1, GW * NCH * (1 + KK)]]))
                        xwing = wpool.tile([128, GW * C], DT, name="xwing",
                                           bufs=2)
                        nc.sync.dma_start(
                            xwing[:],
                            _ap(xown, g * GW * 128 * RL,
                                [[RL, 128], [128 * RL, GW], [1, C]]))
                        out_dt = F32 if last else DT
                        rowsg = wpool.tile([128, GW * C], out_dt,
                                           name="rowsg", bufs=2)

                        if nocompute:
                            nc.vector.memset(rowsg[:], 0)
                        for wl in range(GW) if not nocompute else []:
                            u = wpool.tile([128, NCH * UD], DT, name="u",
                                           bufs=2)
                            for p in range(2):
                                npool = (min(upool_eff, NCHA)
                                         if p == 1 else 0)
                                ndve = NCHA - npool
                                if ndve:
                                    nc.vector.tensor_tensor(
                                        out=_sap(u, p * NCHA * UD,
                                                 [[UD, ndve], [C, KK],
                                                  [1, C]]),
                                        in0=_sap(xjt[p], wl * NCHA * RL,
                                                 [[RL, ndve], [0, KK],
                                                  [1, C]]),
                                        in1=_sap(edt,
                                                 (wl * NCH + p * NCHA)
                                                 * (1 + KK) + 1,
                                                 [[1 + KK, ndve], [1, KK],
                                                  [0, C]]),
                                        op=mybir.AluOpType.mult)
                                if npool:
                                    nc.gpsimd.tensor_tensor(
                                        out=_sap(u, (p * NCHA + ndve) * UD,
                                                 [[UD, npool], [C, KK],
                                                  [1, C]]),
                                        in0=_sap(xjt[p],
                                                 (wl * NCHA + ndve) * RL,
                                                 [[RL, npool], [0, KK],
                                                  [1, C]]),
                                        in1=_sap(edt,
                                                 (wl * NCH + p * NCHA
                                                  + ndve) * (1 + KK) + 1,
                                                 [[1 + KK, npool], [1, KK],
                                                  [0, C]]),
                                        op=mybir.AluOpType.mult)
                            inc = wpool.tile([128, NCH * 128], DT,
                                             name="inc", bufs=2)
                            inc_eng = (nc.gpsimd if cfg.INCPOOL
                                       else nc.vector)
                            inc_eng.tensor_tensor(
                                out=_sap(inc, 0, [[128, NCH], [1, 128]]),
                                in0=_sap(iotaf, 0, [[0, NCH], [1, 128]]),
                                in1=_sap(edt, wl * NCH * (1 + KK),
                                         [[1 + KK, NCH], [0, 128]]),
                                op=mybir.AluOpType.is_equal)

                            z = ppool.tile([128, UD], F32, name="z", bufs=2)
                            for c in range(NCH):
                                # every chunk matmul writes all of z
                                # densely, so no zero-init is needed
                                nc.tensor.matmul(
                                    z[:],
                                    _sap(inc, c * 128, [[1, 128]]),
                                    _sap(u, c * UD, [[1, UD]]),
                                    start=(c == 0), stop=(c == NCH - 1))

                            zsb = wpool.tile([128, UD], F32, name="zsb",
                                             bufs=2)
                            nc.scalar.copy(zsb[:], z[:])
                            zt = ppool.tile([96, 384], F32, name="zt",
                                            bufs=2)
                            for j in range(3):
                                nc.tensor.transpose(
                                    _sap(zt, j * 128, [[1, 128]]),
                                    _sap(zsb, j * 96, [[1, 96]]),
                                    identf[:])
                            ztsb = wpool.tile([96, 384], F32, name="ztsb",
                                              bufs=2)
                            nc.scalar.copy(ztsb[:], zt[:])

                            agg = ppool.tile([O, 128], F32, name="agg",
                                             bufs=1)
                            for j in range(3):
                                nc.tensor.matmul(
                                    agg[:],
                                    _sap(wfsb, j * 32, [[1, 32]]),
                                    _sap(ztsb, j * 128, [[1, 128]]),
                                    start=(j == 0), stop=False)
                            xt = ppool.tile([C, 128], DT, name="xt", bufs=1)
                            nc.tensor.transpose(
                                xt[:],
                                _sap(xwing, wl * C, [[1, C]]),
                                identd[:])
                            xtsb = wpool.tile([C, 128], DT, name="xtsb",
                                              bufs=2)
                            nc.scalar.copy(xtsb[:], xt[:])
                            nc.tensor.matmul(agg[:], rtsb[:], xtsb[:],
                                             start=False, stop=True)
                            ht = wpool.tile([O, 128], out_dt, name="ht",
                                            bufs=2)
                            nc.scalar.activation(
                                ht[:], agg[:],
                                mybir.ActivationFunctionType.Relu,
                                bias=bbsb[:], scale=1.0)
                            rows = ppool.tile([128, O], out_dt, name="rows",
                                              bufs=1)
                            nc.tensor.transpose(
                                rows[:], ht[:],
                                _sap(identf if last else identd, 0,
                                     [[1, 32]], npart=32))
                            nc.scalar.copy(
                                _sap(rowsg, wl * C, [[1, C]]), rows[:])

                        if last:
                            nc.sync.dma_start(
                                _ap(rows_out, g * GW * 128 * C,
                                    [[C, 128], [128 * C, GW], [1, C]]),
                                rowsg[:])
                        else:
                            nc.sync.dma_start(
                                _ap(rows_out, g * GW * 128 * RL,
                                    [[RL, 128], [128 * RL, GW], [1, C]]),
                                rowsg[:])

            for _rep in range(repeat):
                # layer 1: full mesh (both halves), writes the local table
                layer(tab0.ap(), tab0.ap(), wfs[0], rts[0], bbs[0],
                      tfull[:], last=False, ng=2 * NG)
                # layer 2: own half only (first NG groups of the same
                # index/edge arrays), gathers from the local table
                layer(tfull[:], tfull[:], wfs[1], rts[1], bbs[1],
                      outt.ap(), last=True, ng=NG)

    nc.finalize()
    return nc


# ------------------------------------------------------------------- driver

_cache = {}


def _get_program(cfg):
    key = (cfg.NW, cfg.NCHA, cfg.GW, cfg.SG, cfg.NSPLIT, cfg.DT,
           cfg.NCORES, cfg.UPOOL, cfg.INCPOOL)
    if key not in _cache:
        _cache[key] = build_program(cfg, cfg.NCORES)
    return _cache[key]


def run(cfg, images, edges, pseudo, W1, root1, b1, W2, root2, b2,
        trace=False, trace_out=None):
    wf = []
    for W in (W1, W2):
        Wflat = np.asarray(W, np.float32).reshape(cfg.KK * cfg.C, cfg.O)
        wfl = np.zeros((96, 96), np.float32)
        for j in range(3):
            wfl[:, 32 * j:32 * j + 32] = Wflat[96 * j:96 * j + 96, :]
        wf.append(wfl)
    rts = [np.asarray(r, np.float32).astype(_np_dt(cfg.DT))
           for r in (root1, root2)]
    bbs = [np.asarray(b, np.float32).reshape(cfg.O, 1) for b in (b1, b2)]

    in_maps = []
    sigmas = []
    for b in range(cfg.B):
        tab0, cores, sigma = _host_prep_mesh(
            cfg, np.asarray(images[b], np.float32),
            np.asarray(edges[b]), np.asarray(pseudo[b], np.float32))
        sigmas.append(sigma)
        for h in range(2):
            IDXA, IDXB, ED = cores[h]
            in_maps.append({
                "tab0": np.roll(tab0, -h * cfg.NHALF, axis=0),
                "idxa": IDXA, "idxb": IDXB,
                "edd": ED,
                "wf1": wf[0], "wf2": wf[1],
                "rt1": rts[0], "rt2": rts[1],
                "bb1": bbs[0], "bb2": bbs[1],
            })

    nc = _get_program(cfg)
    res = bass_utils.run_bass_kernel_spmd(
        nc, in_maps, core_ids=list(range(cfg.NCORES)), trace=trace)
    if trace_out is not None:
        trace_out.append(res)
    outs = res.results

    out = np.empty((cfg.B, cfg.N, cfg.O), np.float32)
    for b in range(cfg.B):
        full = np.concatenate([outs[2 * b]["out"], outs[2 * b + 1]["out"]],
                              axis=0)
        out[b] = full[sigmas[b]]
    return out


def kernel(images, edges, pseudo, W1, root1, b1, W2, root2, b2):
    cfg = CFG()
    return run(cfg, images, edges, pseudo, W1, root1, b1,
               W2, root2, b2)



# revision 6
# speedup vs baseline: 7.3400x; 7.3400x over previous
"""Trainium2 Bass kernel for nn_MeshDownConv (2-layer SplineConv GNN).

Sharding: 4 cores, one full mesh per core (zero cross-core communication).
The wall-clock of kernel() is dominated by host preprocessing and the
host->device tunnel, so the design minimizes uploaded bytes and Python time:

- Compact uploads per core (~10.5 MB instead of ~45 MB):
    tab0c [NN,32] f16   node features (gather table rows are expanded to the
                        256B dma_gather granule on device)
    idxc  [32,NI16] i16 gather indices, one copy (device replicates x8 into
                        the 128-partition layout SWDGE wants)
    edd3  [128,NCHT,3]  per-edge-slot (dstoff, t0, t1); the 9 B-spline basis
                        values are computed on device in a prepass
- A device prepass expands these into internal-DRAM tensors (texp, idxT,
  edd10); the main loops are unchanged relative to the incidence-matmul
  design: dma_gather source rows, DVE builds u[e,(k,c)] = basis_k*xj_c and
  the one-hot incidence, PE contracts edges into PSUM, node side multiplies
  by Wflat + root + bias + relu.
- Host prep is fully vectorized numpy (snake load-balancer + counting-sort
  style window packing), one thread per mesh.
- Execution uses a module-cached jit(shard_map) so repeat calls skip
  re-tracing and re-compiling the NEFF; inputs are device_put per mesh as
  soon as they are ready; donated output buffers are created on device.
- Output is fetched as f16 [NN,32] per core and cast to f32 on host.
"""
import sys

sys.path.insert(0, "/opt/trn_rl_repo")

import numpy as np

import concourse.bass as bass
import concourse.mybir as mybir
from concourse import bacc, tile

F32 = mybir.dt.float32
F16 = mybir.dt.float16
I16 = mybir.dt.int16
I32 = mybir.dt.int32


class CFG:
    C = 32            # in channels
    O = 32            # out channels
    KK = 9            # spline kernels
    NW = 196          # windows per table half (i16 index range)
    NCHA = 9          # chunks (of 128 edges) per window per pass
    GW = 4            # windows per gather group
    SG = 8            # groups per idx-streaming supergroup
    NSPLIT = 2        # gather pieces per (group, pass)
    N = 50000         # real nodes per mesh
    E = 800000        # edges per mesh
    B = 4             # meshes
    NCORES = 4        # one mesh per core
    DT = F16

    @property
    def RL(self):
        return 64 if self.DT == F32 else 128   # 256B gather granule

    @property
    def NCH(self):
        return 2 * self.NCHA

    @property
    def NHALF(self):
        return self.NW * 128

    @property
    def NN(self):
        return 2 * self.NHALF

    @property
    def NWT(self):
        return 2 * self.NW            # windows per mesh (both layers)

    @property
    def NCHT(self):
        return self.NWT * self.NCH

    @property
    def NSLOTT(self):
        return self.NWT * self.NCHA * 128

    @property
    def NI16(self):
        return self.NSLOTT // 16


def _np_dt(dt):
    return {F32: np.float32, F16: np.float16}[dt]


# ----------------------------------------------------------------- host prep

def _snake_sigma(deg, nbins, cap):
    """Balanced node->slot permutation: sort by degree desc, snake over bins.
    sigma[node] = bin*cap + slot."""
    n = deg.shape[0]
    order = np.argsort(-deg, kind="stable")
    r = np.arange(n, dtype=np.int64)
    row = r // nbins
    col = r % nbins
    binidx = np.where(row % 2 == 0, col, nbins - 1 - col)
    sigma = np.empty(n, np.int64)
    sigma[order] = binidx * cap + row
    return sigma


def _balance_nodes_greedy(deg, nbins, cap_nodes=128):
    import heapq
    n = deg.shape[0]
    order = np.argsort(-deg, kind="stable")
    heap = [(0, b) for b in range(nbins)]
    heapq.heapify(heap)
    counts = np.zeros(nbins, np.int64)
    sums = np.zeros(nbins, np.int64)
    sigma = np.empty(n, np.int64)
    for old in order:
        while True:
            s, b = heapq.heappop(heap)
            if counts[b] < cap_nodes:
                break
        sigma[old] = b * 128 + counts[b]
        counts[b] += 1
        sums[b] += deg[old]
        if counts[b] < cap_nodes:
            heapq.heappush(heap, (sums[b], b))
    return sigma


def _host_prep_mesh(cfg, x, edge, pseudo):
    """Vectorized per-mesh host preprocessing.

    Returns (tab0c [NN,32] f16, IDXC [32,NI16] i16, ED3 [128,NCHT,3] f16,
    sigma [N] node->row permutation)."""
    npdt = _np_dt(cfg.DT)
    src = np.ascontiguousarray(edge[0], np.int32)
    dst = np.ascontiguousarray(edge[1], np.int32)
    E = src.shape[0]
    capa = cfg.NCHA * 128

    deg = np.bincount(dst, minlength=cfg.N)
    for attempt in range(2):
        if attempt == 0:
            sigma = _snake_sigma(deg, cfg.NWT, 128).astype(np.int32)
        else:
            sigma = _balance_nodes_greedy(deg, cfg.NWT).astype(np.int32)
        gsrc = sigma[src]
        gdst = sigma[dst]
        # key = window(dst)*2 + srcpass fits int16 (< 2*NWT = 784)
        key = ((gdst >> 7) * 2 + (gsrc >= cfg.NHALF)).astype(np.int16)
        order = np.argsort(key, kind="stable")   # radix on int16
        sk = key[order].astype(np.int32)
        starts = np.searchsorted(sk, np.arange(2 * cfg.NWT + 1)).astype(
            np.int32)
        slot = np.arange(E, dtype=np.int32) - starts[sk]
        if slot.max() < capa:
            break
    else:
        raise RuntimeError("window capacity exceeded")

    w = sk >> 1
    pp = sk & 1

    # per-slot edge data; precompute f16 source columns, then gather
    dlow = (gdst & 127).astype(npdt)
    ps16 = pseudo.astype(npdt)
    ED3 = np.zeros((128, cfg.NCHT, 3), npdt)
    ED3[:, :, 0] = 128.0          # sentinel: empty slots match no node lane
    ev = np.empty((E, 3), npdt)
    ev[:, 0] = dlow[order]
    ev[:, 1:] = ps16[order]
    ch = pp * cfg.NCHA + w * cfg.NCH + (slot >> 7)
    ED3[slot & 127, ch] = ev

    # gather indices: linear per-pass layout, then reshape-transpose into
    # the 16-partition-wrapped layout SWDGE wants
    gslot = w * capa + slot
    vals = (gsrc[order] - pp * cfg.NHALF).astype(np.int16)
    IDXC = np.zeros((2, cfg.NSLOTT), np.int16)
    IDXC[pp, gslot] = vals
    IDXC = np.ascontiguousarray(
        IDXC.reshape(2, cfg.NI16, 16).transpose(0, 2, 1)).reshape(
        32, cfg.NI16)

    tab0c = np.zeros((cfg.NN, cfg.C), npdt)
    tab0c[sigma] = x.astype(npdt)
    return tab0c, IDXC, ED3, sigma


# ------------------------------------------------------------- bass program

def _ap(t, offset, pattern):
    base = t if isinstance(t, bass.AP) else t[:]
    return bass.AP(base.tensor, base.offset + offset, pattern)


def _sap(t, offset, freedims, npart=None):
    base = t if isinstance(t, bass.AP) else t[:]
    p = base.ap[0]
    part = [p[0], p[1] if npart is None else npart]
    return bass.AP(base.tensor, base.offset + offset, [part] + list(freedims))


def build_program(cfg, ncores):
    nc = bacc.Bacc("TRN2", target_bir_lowering=False, debug=False,
                   num_devices=ncores)
    DT = cfg.DT
    C, O, KK, RL = cfg.C, cfg.O, cfg.KK, cfg.RL
    NCH, NCHA, GW = cfg.NCH, cfg.NCHA, cfg.GW
    NI16, NCHT, NN, NHALF = cfg.NI16, cfg.NCHT, cfg.NN, cfg.NHALF
    NG = cfg.NWT // GW            # 98 groups cover the whole mesh
    UD = KK * C                   # 288

    tab0c = nc.dram_tensor("tab0c", [NN, C], DT, kind="ExternalInput")
    idxc = nc.dram_tensor("idxc", [32, NI16], I16, kind="ExternalInput")
    edd3 = nc.dram_tensor("edd3", [128, NCHT, 3], DT, kind="ExternalInput")
    wf1 = nc.dram_tensor("wf1", [96, 96], F32, kind="ExternalInput")
    wf2 = nc.dram_tensor("wf2", [96, 96], F32, kind="ExternalInput")
    rt1 = nc.dram_tensor("rt1", [C, O], DT, kind="ExternalInput")
    rt2 = nc.dram_tensor("rt2", [C, O], DT, kind="ExternalInput")
    bb1 = nc.dram_tensor("bb1", [O, 1], F32, kind="ExternalInput")
    bb2 = nc.dram_tensor("bb2", [O, 1], F32, kind="ExternalInput")
    outt = nc.dram_tensor("out", [NN, C], DT, kind="ExternalOutput")

    with tile.TileContext(nc, num_cores=ncores) as tc:
        with tc.tile_pool(name="const", bufs=1) as cpool, \
             tc.tile_pool(name="dram", bufs=1, space="DRAM") as dpool, \
             tc.tile_pool(name="work", bufs=2) as wpool, \
             tc.tile_pool(name="psum", bufs=1, space="PSUM") as ppool:

            from concourse import library_config
            nc.gpsimd.load_library(library_config.mlp)

            # ---- constants
            iotai = cpool.tile([128, 128], I32, name="iotai")
            nc.gpsimd.iota(iotai[:], pattern=[[1, 128]], base=0,
                           channel_multiplier=0)
            iotaf = cpool.tile([128, 128], DT, name="iotaf")
            nc.vector.tensor_copy(iotaf[:], iotai[:])
            idximp = cpool.tile([128, 128], I32, name="idximp")
            nc.gpsimd.iota(idximp[:], pattern=[[1, 128]], base=0,
                           channel_multiplier=-1)
            identf = cpool.tile([128, 128], F32, name="identf")
            nc.vector.tensor_scalar(out=identf[:], in0=idximp[:],
                                    scalar1=0, scalar2=None,
                                    op0=mybir.AluOpType.is_equal)
            identd = identf
            if DT != F32:
                identd = cpool.tile([128, 128], DT, name="identd")
                nc.vector.tensor_copy(identd[:], identf[:])

            wfs = [cpool.tile([96, 96], F32, name=f"wfs{i}") for i in range(2)]
            rts = [cpool.tile([C, O], DT, name=f"rts{i}") for i in range(2)]
            bbs = [cpool.tile([O, 1], F32, name=f"bbs{i}") for i in range(2)]
            for i, (wsrc, rsrc, bsrc) in enumerate(
                    [(wf1, rt1, bb1), (wf2, rt2, bb2)]):
                nc.sync.dma_start(wfs[i][:], wsrc[:])
                nc.sync.dma_start(rts[i][:], rsrc[:])
                nc.sync.dma_start(bbs[i][:], bsrc[:])

            # ---- internal DRAM
            texp = dpool.tile([NN, RL], DT, name="texp")
            tfull = dpool.tile([NN, RL], DT, name="tfull")
            idxT = dpool.tile([128, 2 * NI16], I16, name="idxT")
            edd10 = dpool.tile([128, NCHT * 10], DT, name="edd10")

            # ---- prepass 1: expand the node table to the 256B granule
            nc.sync.dma_start(_ap(texp, 0, [[RL, NN], [1, C]]), tab0c[:])

            # ---- prepass 2: replicate gather indices x8 into 128 partitions
            for pp in range(2):
                for k in range(8):
                    nc.sync.dma_start(
                        _ap(idxT, k * 16 * (2 * NI16) + pp * NI16,
                            [[2 * NI16, 16], [1, NI16]]),
                        _ap(idxc.ap(), pp * 16 * NI16,
                            [[NI16, 16], [1, NI16]]))

            # ---- prepass 3: expand (dstoff,t0,t1) -> (dstoff, 9 basis vals)
            SQH = float(np.sqrt(0.5))
            bias_sq = cpool.tile([128, 1], F32, name="bias_sq")
            nc.vector.memset(bias_sq[:], SQH)
            bias_z = cpool.tile([128, 1], F32, name="bias_z")
            nc.vector.memset(bias_z[:], 0.0)
            NCHUNK = 16
            CC = NCHT // NCHUNK
            for ci in range(NCHUNK):
                e3 = wpool.tile([128, CC * 3], DT, name="pe3", bufs=2)
                nc.sync.dma_start(
                    e3[:], _ap(edd3.ap(), ci * CC * 3,
                               [[NCHT * 3, 128], [1, CC * 3]]))
                bts = []
                for d in range(2):
                    bt = wpool.tile([128, CC * 3], DT, name=f"pb{d}", bufs=2)
                    tv = _sap(e3, 1 + d, [[3, CC]])
                    # open quadratic B-spline pieces of t in [0,1):
                    # B0 = 0.5(1-t)^2, B2 = 0.5 t^2, B1 = 1 - B0 - B2
                    nc.scalar.activation(
                        _sap(bt, 0, [[3, CC]]), tv,
                        mybir.ActivationFunctionType.Square,
                        bias=bias_sq[:], scale=-SQH)
                    nc.scalar.activation(
                        _sap(bt, 2, [[3, CC]]), tv,
                        mybir.ActivationFunctionType.Square,
                        bias=bias_z[:], scale=SQH)
                    tmp = wpool.tile([128, CC], DT, name=f"pt{d}", bufs=2)
                    nc.vector.tensor_tensor(
                        out=tmp[:], in0=_sap(bt, 0, [[3, CC]]),
                        in1=_sap(bt, 2, [[3, CC]]),
                        op=mybir.AluOpType.add)
                    nc.vector.tensor_scalar(
                        out=_sap(bt, 1, [[3, CC]]), in0=tmp[:],
                        scalar1=-1.0, scalar2=1.0,
                        op0=mybir.AluOpType.mult,
                        op1=mybir.AluOpType.add)
                    bts.append(bt)
                e10 = wpool.tile([128, CC * 10], DT, name="pe10", bufs=2)
                nc.vector.tensor_copy(_sap(e10, 0, [[10, CC]]),
                                      _sap(e3, 0, [[3, CC]]))
                # basis[c, 3j+i] = B1[c,j] * B0[c,i]
                nc.vector.tensor_tensor(
                    out=_sap(e10, 1, [[10, CC], [3, 3], [1, 3]]),
                    in0=_sap(bts[0], 0, [[3, CC], [0, 3], [1, 3]]),
                    in1=_sap(bts[1], 0, [[3, CC], [1, 3], [0, 3]]),
                    op=mybir.AluOpType.mult)
                nc.sync.dma_start(
                    _ap(edd10, ci * CC * 10, [[NCHT * 10, 128], [1, CC * 10]]),
                    e10[:])

            nsg = GW * NCHA * 128          # gather idxs per (group, pass)
            npiece = nsg // cfg.NSPLIT
            nchp = GW * NCHA // cfg.NSPLIT  # gathered chunks per piece

            def layer(tabsrc, xown, xrl, wfsb, rtsb, bbsb, rows_out, orl,
                      last):
                for g0 in range(0, NG, cfg.SG):
                    gcnt = min(cfg.SG, NG - g0)
                    sidxt = [None, None]
                    for pdx in range(2):
                        st = wpool.tile([128, cfg.SG * nsg // 16], I16,
                                        name=f"sidx{pdx}", bufs=2)
                        nc.sync.dma_start(
                            _sap(st, 0, [[1, gcnt * nsg // 16]]),
                            _ap(idxT, pdx * NI16 + g0 * nsg // 16,
                                [[2 * NI16, 128], [1, gcnt * nsg // 16]]))
                        sidxt[pdx] = st
                    for gl in range(gcnt):
                        g = g0 + gl
                        xjt = []
                        for pdx in range(2):
                            xj = wpool.tile([128, GW * NCHA * RL], DT,
                                            name=f"xj{pdx}", bufs=2)
                            for s in range(cfg.NSPLIT):
                                nc.gpsimd.dma_gather(
                                    out_ap=_sap(xj, s * nchp * RL,
                                                [[RL, nchp], [1, RL]]),
                                    in_ap=_ap(tabsrc, pdx * NHALF * RL,
                                              [[RL, NHALF], [1, RL]]),
                                    idxs_ap=_sap(
                                        sidxt[pdx],
                                        (gl * nsg + s * npiece) // 16,
                                        [[1, npiece // 16]]),
                                    num_idxs=npiece,
                                    num_idxs_reg=npiece,
                                    elem_size=RL,
                                    single_packet=False,
                                )
                            xjt.append(xj)
                        edt = wpool.tile([128, GW * NCH * 10], DT,
                                         name="edt", bufs=2)
                        nc.sync.dma_start(
                            edt[:],
                            _ap(edd10, g * GW * NCH * 10,
                                [[NCHT * 10, 128], [1, GW * NCH * 10]]))
                        xwing = wpool.tile([128, GW * C], DT, name="xwing",
                                           bufs=2)
                        nc.sync.dma_start(
                            xwing[:],
                            _ap(xown, g * GW * 128 * xrl,
                                [[xrl, 128], [128 * xrl, GW], [1, C]]))
                        rowsg = wpool.tile([128, GW * C], DT, name="rowsg",
                                           bufs=2)

                        for wl in range(GW):
                            u = wpool.tile([128, NCH * UD], DT, name="u",
                                           bufs=2)
                            for pdx in range(2):
                                nc.vector.tensor_tensor(
                                    out=_sap(u, pdx * NCHA * UD,
                                             [[UD, NCHA], [C, KK], [1, C]]),
                                    in0=_sap(xjt[pdx], wl * NCHA * RL,
                                             [[RL, NCHA], [0, KK], [1, C]]),
                                    in1=_sap(edt,
                                             (wl * NCH + pdx * NCHA) * 10 + 1,
                                             [[10, NCHA], [1, KK], [0, C]]),
                                    op=mybir.AluOpType.mult)
                            inc = wpool.tile([128, NCH * 128], DT,
                                             name="inc", bufs=2)
                            nc.vector.tensor_tensor(
                                out=_sap(inc, 0, [[128, NCH], [1, 128]]),
                                in0=_sap(iotaf, 0, [[0, NCH], [1, 128]]),
                                in1=_sap(edt, wl * NCH * 10,
                                         [[10, NCH], [0, 128]]),
                                op=mybir.AluOpType.is_equal)

                            z = ppool.tile([128, UD], F32, name="z", bufs=2)
                            for c in range(NCH):
                                nc.tensor.matmul(
                                    z[:],
                                    _sap(inc, c * 128, [[1, 128]]),
                                    _sap(u, c * UD, [[1, UD]]),
                                    start=(c == 0), stop=(c == NCH - 1))

                            zsb = wpool.tile([128, UD], F32, name="zsb",
                                             bufs=2)
                            nc.scalar.copy(zsb[:], z[:])
                            zt = ppool.tile([96, 384], F32, name="zt",
                                            bufs=2)
                            for j in range(3):
                                nc.tensor.transpose(
                                    _sap(zt, j * 128, [[1, 128]]),
                                    _sap(zsb, j * 96, [[1, 96]]),
                                    identf[:])
                            ztsb = wpool.tile([96, 384], F32, name="ztsb",
                                              bufs=2)
                            nc.scalar.copy(ztsb[:], zt[:])

                            agg = ppool.tile([O, 128], F32, name="agg",
                                             bufs=1)
                            for j in range(3):
                                nc.tensor.matmul(
                                    agg[:],
                                    _sap(wfsb, j * 32, [[1, 32]]),
                                    _sap(ztsb, j * 128, [[1, 128]]),
                                    start=(j == 0), stop=False)
                            xt = ppool.tile([C, 128], DT, name="xt", bufs=1)
                            nc.tensor.transpose(
                                xt[:],
                                _sap(xwing, wl * C, [[1, C]]),
                                identd[:])
                            xtsb = wpool.tile([C, 128], DT, name="xtsb",
                                              bufs=2)
                            nc.scalar.copy(xtsb[:], xt[:])
                            nc.tensor.matmul(agg[:], rtsb[:], xtsb[:],
                                             start=False, stop=True)
                            ht = wpool.tile([O, 128], DT, name="ht",
                                            bufs=2)
                            nc.scalar.activation(
                                ht[:], agg[:],
                                mybir.ActivationFunctionType.Relu,
                                bias=bbsb[:], scale=1.0)
                            rows = ppool.tile([128, O], DT, name="rows",
                                              bufs=1)
                            nc.tensor.transpose(
                                rows[:], ht[:],
                                _sap(identd, 0, [[1, 32]], npart=32))
                            nc.scalar.copy(
                                _sap(rowsg, wl * C, [[1, C]]), rows[:])

                        nc.sync.dma_start(
                            _ap(rows_out, g * GW * 128 * orl,
                                [[orl, 128], [128 * orl, GW], [1, C]]),
                            rowsg[:])

            # layer 1: gathers from texp, root term from compact tab0c,
            # writes the local full table
            layer(texp[:], tab0c.ap(), C, wfs[0], rts[0], bbs[0],
                  tfull[:], RL, last=False)
            # layer 2: gathers from tfull, writes the compact f16 output
            layer(tfull[:], tfull[:], RL, wfs[1], rts[1], bbs[1],
                  outt.ap(), C, last=True)

    nc.finalize()
    return nc


# ------------------------------------------------------------------- runner

_RT = None


def _get_runtime(cfg):
    global _RT
    if _RT is not None:
        return _RT

    import jax
    import jax.numpy as jnp
    from jax.sharding import Mesh, PartitionSpec, NamedSharding
    from jax.experimental.shard_map import shard_map
    from concourse.bass2jax import (_bass_exec_p, install_neuronx_cc_hook,
                                    partition_id_tensor)

    try:
        jax.config.update("jax_compilation_cache_dir", "/tmp/meshconv_jaxcache")
        jax.config.update("jax_persistent_cache_min_compile_time_secs", 0.5)
    except Exception:
        pass

    nc = build_program(cfg, cfg.NCORES)
    install_neuronx_cc_hook()

    partition_name = (nc.partition_id_tensor.name
                      if nc.partition_id_tensor else None)
    in_names, out_names, out_avals = [], [], []
    for alloc in nc.m.functions[0].allocations:
        if not isinstance(alloc, mybir.MemoryLocationSet):
            continue
        name = alloc.memorylocations[0].name
        if alloc.kind == "ExternalInput":
            if name != partition_name:
                in_names.append(name)
        elif alloc.kind == "ExternalOutput":
            out_names.append(name)
            out_avals.append(jax.core.ShapedArray(
                tuple(alloc.tensor_shape), mybir.dt.np(alloc.dtype)))
    n_params = len(in_names)
    n_outs = len(out_names)
    all_names = list(in_names) + list(out_names)
    if partition_name is not None:
        all_names.append(partition_name)
    donate = tuple(range(n_params, n_params + n_outs))

    n_cores = cfg.NCORES
    devices = jax.devices()[:n_cores]
    mesh = Mesh(np.asarray(devices), ("core",))
    spec = NamedSharding(mesh, PartitionSpec("core"))

    def _body(*args):
        operands = list(args)
        if partition_name is not None:
            operands.append(partition_id_tensor())
        outs = _bass_exec_p.bind(
            *operands,
            out_avals=tuple(out_avals),
            in_names=tuple(all_names),
            out_names=tuple(out_names),
            lowering_input_output_aliases=(),
            sim_require_finite=True,
            sim_require_nnan=True,
            nc=nc,
        )
        return tuple(outs)

    sharded = jax.jit(
        shard_map(_body, mesh=mesh,
                  in_specs=(PartitionSpec("core"),) * (n_params + n_outs),
                  out_specs=(PartitionSpec("core"),) * n_outs,
                  check_rep=False),
        donate_argnums=donate, keep_unused=True)

    def _zeros():
        return tuple(
            jnp.zeros((n_cores * a.shape[0], *a.shape[1:]), a.dtype)
            for a in out_avals)
    zeros_fn = jax.jit(_zeros, out_shardings=(spec,) * n_outs)

    _RT = dict(nc=nc, jax=jax, mesh=mesh, spec=spec, devices=devices,
               in_names=in_names, out_names=out_names, out_avals=out_avals,
               sharded=sharded, zeros_fn=zeros_fn)
    return _RT


def run(cfg, images, edges, pseudo, W1, root1, b1, W2, root2, b2,
        trace=False, trace_out=None):
    rt = _get_runtime(cfg)
    jax = rt["jax"]
    devices = rt["devices"]
    npdt = _np_dt(cfg.DT)

    wf = []
    for W in (W1, W2):
        Wflat = np.asarray(W, np.float32).reshape(cfg.KK * cfg.C, cfg.O)
        wfl = np.zeros((96, 96), np.float32)
        for j in range(3):
            wfl[:, 32 * j:32 * j + 32] = Wflat[96 * j:96 * j + 96, :]
        wf.append(wfl)
    rts = [np.asarray(r, np.float32).astype(npdt) for r in (root1, root2)]
    bbs = [np.asarray(b, np.float32).reshape(cfg.O, 1) for b in (b1, b2)]
    const = {"wf1": wf[0], "wf2": wf[1], "rt1": rts[0], "rt2": rts[1],
             "bb1": bbs[0], "bb2": bbs[1]}

    # host prep per mesh, device_put (async) as soon as each mesh is ready
    pieces = {n: [None] * cfg.NCORES for n in rt["in_names"]}
    sigmas = []
    for b in range(cfg.B):
        tab0c, IDXC, ED3, sigma = _host_prep_mesh(
            cfg, np.asarray(images[b], np.float32),
            np.asarray(edges[b]), np.asarray(pseudo[b], np.float32))
        sigmas.append(sigma)
        per = {"tab0c": tab0c, "idxc": IDXC, "edd3": ED3, **const}
        for n in rt["in_names"]:
            pieces[n][b] = jax.device_put(per[n], devices[b])

    globals_in = []
    for n in rt["in_names"]:
        shards = pieces[n]
        shape0 = shards[0].shape
        garr = jax.make_array_from_single_device_arrays(
            (cfg.NCORES * shape0[0], *shape0[1:]), rt["spec"], shards)
        globals_in.append(garr)
    zeros = rt["zeros_fn"]()

    out_arrs = rt["sharded"](*globals_in, *zeros)
    outg = np.asarray(out_arrs[0]).reshape(cfg.NCORES, cfg.NN, cfg.C)

    out = np.empty((cfg.B, cfg.N, cfg.O), np.float32)
    for b in range(cfg.B):
        out[b] = outg[b].astype(np.float32)[sigmas[b]]
    return out


def kernel(images, edges, pseudo, W1, root1, b1, W2, root2, b2):
    cfg = CFG()
    return run(cfg, images, edges, pseudo, W1, root1, b1,
               W2, root2, b2)
